# revision 15
# baseline (speedup 1.0000x reference)
"""BinaryMatchAttention Trainium2 kernel.

reference semantics (per batch b):
    qb[k]   = (query_addr >> k) & 1                 k in [0, 16)
    w[s]    = prod_k (1 - |x[b, s, 96+k] - qb[k]|)
    out[b,d]= sum_s w[s] * x[b, s, d]               d in [0, 96)

Sharding: data-parallel over batch, one NeuronCore per batch element
(B == 8 == n_cores), no collectives.

Per-core plan (x_core [32768, 128] fp32 in HBM, memory-bound):
  - flat row split: partition p holds the 256 consecutive seq rows
    s = 256p + i; DMA waves load i-ranges for all partitions, split
    across the two HWDGE rings (Sync / ACT) with explicit per-ring
    byte balance.
  - THROTTLE-AWARE MIXED-WIDTH STREAM: full 512B-row waves move at
    ~415GB/s aggregate, but ~16.5MB at that rate trips a hardware DMA
    throttle (DMA clamps to ~25GB/s and the remainder crawls — worth
    ~7us).  448B-row waves (cols 0:112, all the kernel reads; one
    descriptor per row) are descriptor-rate-bound at ~310GB/s and
    never trip it.  ~28% of rows go as 448B waves, cutting total
    traffic to ~16.1MB — under the throttle budget — so the stream
    finishes at full speed regardless of device throttle state.
  - ring-start equalizer: the DMA engines drain the first-doorbelled
    ring's opening batch before visiting the other ring (~2-3us head
    start); a tiny 128x64B-descriptor cq load at the head of EACH ring
    costs ~0.4us and evens the starts.  Sync's copy is the real query
    tile, ACT's is a dummy.
  - one completion semaphore per DMA: a rotating slot pool makes tail
    waves' triggers wait on mid-stream receipts, starving the rings at
    the end; unique sems let every trigger issue back-to-back so the
    rings hold their full descriptor backlog from ~13us on.
  - match weights per wave on DVE: d = bits - qb, na = min(-d, d),
    t = 1 + na = 1 - |d|, then 4 strided pairwise products 16 -> 1.
    Chains run in interleaved pairs to hide the ~150ns sem-update hop;
    16-row waves keep the fixed ~100ns/op overhead amortized, with one
    4-row closer per ring so the final chain is short.
  - einsum on TensorE: per 4-row group, psum[4, 384] += w4.T @ v[4, 96]
    (diagonal trick: only r==r' 96-blocks are wanted; host extracts).
    One PSUM accumulator across all 64 groups, then PSUM -> SBUF ->
    HBM (DMA cannot read PSUM on this stack).
  - no manual end-of-kernel sem restore by default: the NEFF exit
    protocol itself zeroes every engine's full semaphore file between
    iterations (~250 EVENT_SEMAPHORE instrs, ~7us, barrier-gated after
    Sync's last instruction — it dominates the post-kernel critical
    path and is insensitive to walrus --max-sem-num), which makes our
    own clears redundant.  BMA_CLEAR=1 restores the explicit
    wait+drain+clear; BMA_MAXSEM=<n> passes --max-sem-num to walrus.
  - float32r (TF32-like PE path) gives ~5e-4 rel err; BMA_MM_MODE=f32
    selects the exact-fp32 fallback.
"""

import os
import sys

if "/opt/trn_rl_repo" not in sys.path:
    sys.path.insert(0, "/opt/trn_rl_repo")

import numpy as np

S, D = 32768, 128
VD = 96          # value payload dims
NBITS = 16
BIT0 = 96
P = 128          # partitions
R = 4            # rows per matmul group (diagonal trick)
C = R
IPP = S // P     # 256 rows per partition

# Wave sizes (rows/partition), strictly alternating Sync/ACT rings so
# both rings carry the identical [8, 16x6, 8, 8, 4, 4] = 128-row shape
# and finish together.  Both rings ramp down to 4-row waves so the final
# weight chains and matmuls after the last byte are short.  (The HBM
# stream trips a hardware DMA throttle after ~40us at the ~415GB/s
# saturation rate — DMA clamps to ~25GB/s — so every wave that lands
# after that cliff is catastrophically slow: the whole schedule is
# built to finish the stream by then.)
# Sync's ring gets served ~1.8us before ACT's at stream start (the DMA
# engines drain the first-doorbelled ring's opening batch first), so
# Sync carries 8 extra rows: both rings then END together at ~48us.
# Waves open at 16 rows (one 8KB descriptor per partition) to feed all
# 16 engines from the first trigger.
# Mixed-width stream: full 512B rows move at ~415GB/s but sustained
# ~415GB/s trips a hardware DMA throttle after ~16.5MB/40us (DMA then
# clamps to ~25GB/s and the remainder crawls).  448B rows (cols 0:112,
# all the kernel reads) are descriptor-rate-bound at ~310GB/s and never
# trip it.  Mixing ~28% 448B waves keeps total traffic at ~16.1MB —
# under the throttle budget — so the stream finishes at full speed in
# either device state.  (rows, cols) per wave, interleaved S/A:
# Ring tails are consolidated (A ends with one 12-row 448B wave, not
# 8+4) so the post-stream DVE chain backlog is minimal, and the last
# three waves get SOLO chain groups ordered by landing time — an
# interleaved pair couples a landed wave's ops to its partner's DMA.
_S_WAVES = [(16, 128), (16, 128), (16, 112), (16, 128), (16, 128),
            (16, 112), (16, 128), (16, 128), (4, 112)]   # 132 rows
_A_WAVES = [(16, 128), (16, 128), (16, 112), (16, 128), (16, 128),
            (16, 112), (16, 128), (12, 112)]             # 124 rows
# indices 0..13: S0,A0..S6,A6 interleaved; 14: S7(16f); 15: S8(4c);
# 16: A7(12c, lands last -> PE stop wave)
_WAVES = [w for pair in zip(_S_WAVES[:7], _A_WAVES[:7]) for w in pair]
_WAVES += [_S_WAVES[7], _S_WAVES[8], _A_WAVES[7]]
WROWS = [r for r, c in _WAVES]
WCOLSL = [c for r, c in _WAVES]
assert sum(WROWS) == IPP
# True -> Sync ring, False -> ACT(Scalar) ring
WQ = [k % 2 == 0 for k in range(14)] + [True, True, False]

NCORES = 8

# "f32r" : float32r matmuls (1 cycle/row, ~5e-4 rel err)
# "f32"  : plain fp32 matmuls (4 cycles/row, exact)
MM_MODE = os.environ.get("BMA_MM_MODE", "f32r")

_CACHE = {}


def _build_raw(mode):
    import concourse.bacc as bacc
    import concourse.mybir as mybir

    maxsem = os.environ.get("BMA_MAXSEM")
    if maxsem:
        import concourse.bass_utils as _bu
        if not getattr(_bu, "_bma_maxsem_patched", None):
            _orig_gwa = _bu.get_walrus_args

            def _gwa(arch, tmpdir, *, dve_root=None):
                return _orig_gwa(arch, tmpdir, dve_root=dve_root) + [
                    f"--max-sem-num={int(maxsem)}"
                ]

            _bu.get_walrus_args = _gwa
            _bu._bma_maxsem_patched = True

    f32 = mybir.dt.float32
    x_dt = mybir.dt.float32r if mode == "f32r" else f32
    # DMA cannot read PSUM on this stack (dma_start asserts SBUF/DRAM src)
    out_psum = os.environ.get("BMA_OUT", "sbuf") == "psum"

    nc = bacc.Bacc("TRN2", target_bir_lowering=False, debug=False)
    x = nc.dram_tensor("x", [S, D], x_dt, kind="ExternalInput")
    cq = nc.dram_tensor("cq", [P, NBITS], f32, kind="ExternalInput")
    out = nc.dram_tensor("out", [C, C * VD], f32, kind="ExternalOutput")

    xr = x.ap().rearrange("(p i) d -> p i d", p=P)

    nw = len(WROWS)

    # One DMA-completion sem per transfer.  Rotating a small slot pool
    # is a trap: a slot-reuse wait on a mid-stream receipt throttles the
    # tail waves' triggers to the stream's own pace, starving the HWDGE
    # rings at the end (measured: last 1.6MB dribbled at ~120GB/s).
    # Unique sems let every trigger issue back-to-back at kernel start,
    # so the rings hold the full descriptor backlog from ~13us on.
    dsems = [nc.alloc_semaphore(f"dma{i}") for i in range(nw + 3)]
    duses = [0] * len(dsems)
    # One cumulative DVE-progress sem: engines are pipelined, so even
    # same-engine consumers must wait on the producer's sem update.
    semDVE = nc.alloc_semaphore("dveprog")
    semPE = nc.alloc_semaphore("pedone")

    def dma(eng, dst, src, slot):
        if duses[slot]:
            # slot reuse: order the two uses so a later DMA's increments
            # can never satisfy an earlier DMA's wait target
            eng.wait_ge(dsems[slot], 16 * duses[slot])
        duses[slot] += 1
        eng.dma_start(dst, src).then_inc(dsems[slot], 16)
        return dsems[slot], 16 * duses[slot]

    cqt = nc.alloc_sbuf_tensor("cqt", [P, 1, NBITS], f32)
    cqt2 = nc.alloc_sbuf_tensor("cqt2", [P, 1, NBITS], f32)
    vts = [
        nc.alloc_sbuf_tensor(f"vt{k}", [P, nr, WCOLSL[k]], x_dt)
        for k, nr in enumerate(WROWS)
    ]
    wk = {
        tag: [
            nc.alloc_sbuf_tensor(f"{tag}{i}", [P, max(WROWS), n], f32)
            for i in range(3)
        ]
        for tag, n in (
            ("d", NBITS), ("na", NBITS), ("t", NBITS),
            ("p8", 8), ("p4", 4), ("p2", 2),
        )
    }
    wts = [
        nc.alloc_sbuf_tensor(f"w{k}", [P, nr, 1], x_dt)
        for k, nr in enumerate(WROWS)
    ]
    acc = nc.alloc_psum_tensor("acc", [C, C * VD], f32)
    res = None if out_psum else nc.alloc_sbuf_tensor("res", [C, C * VD], f32)

    # --- ring-start equalizer: the DMA engines serve a ring's pending
    # descriptor batch before visiting the other ring, so whichever ring
    # rings its doorbell first gets a ~3us data head start.  A tiny
    # (128 x 64B descriptor) cq load at the head of EACH ring costs
    # ~0.4us of engine time and puts both rings' first waves on equal
    # footing.  The Sync copy is the real cqt; the ACT copy is a dummy.
    cqr = cq.ap().rearrange("p (a k) -> p a k", a=1)
    cq_sem, cq_tgt = dma(nc.sync, cqt.ap(), cqr, nw + 1)
    dma(nc.scalar, cqt2.ap(), cqr, nw + 2)

    # --- main stream: explicit queue assignment (see WQ) ---
    wave_done = []
    i0 = 0
    for k, nr in enumerate(WROWS):
        eng = nc.sync if WQ[k] else nc.scalar
        wave_done.append(
            dma(eng, vts[k].ap(), xr[:, i0 : i0 + nr, 0 : WCOLSL[k]], k)
        )
        i0 += nr

    # --- DVE: per-wave weight chain.  Every DVE op waits on its
    # predecessor's semDVE update (pipelined engine: program order alone
    # does not order SBUF reads after prior writes).  Tail waves' chains
    # run FIRST (their bits are prefetched), so the last weight is ready
    # mid-stream. ---
    dcnt = 0

    def dve(inst):
        nonlocal dcnt
        dcnt += 1
        inst.then_inc(semDVE, 1)
        return dcnt

    w_ready = [None] * nw

    def chain_ops(k):
        nr = WROWS[k]
        bits = vts[k].ap()[:, :, BIT0 : BIT0 + NBITS]
        bufs = wk
        bi = k % 3
        if mode == "f32r":
            bits = bits.bitcast(f32)
        d = bufs["d"][bi].ap()[:, 0:nr, :]
        na = bufs["na"][bi].ap()[:, 0:nr, :]
        t = bufs["t"][bi].ap()[:, 0:nr, :]
        p8 = bufs["p8"][bi].ap()[:, 0:nr, :]
        p4 = bufs["p4"][bi].ap()[:, 0:nr, :]
        p2 = bufs["p2"][bi].ap()[:, 0:nr, :]
        w = wts[k].ap()
        yield lambda: dve(
            nc.vector.tensor_sub(d, bits, cqt.ap().broadcast_to([P, nr, NBITS]))
        )
        yield lambda: dve(nc.vector.scalar_tensor_tensor(
            na, d, -1.0, d, op0=mybir.AluOpType.mult, op1=mybir.AluOpType.min
        ))
        yield lambda: dve(
            nc.vector.tensor_scalar(t, na, 1.0, None, op0=mybir.AluOpType.add)
        )
        yield lambda: dve(nc.vector.tensor_mul(p8, t[:, :, 0::2], t[:, :, 1::2]))
        yield lambda: dve(nc.vector.tensor_mul(p4, p8[:, :, 0::2], p8[:, :, 1::2]))
        yield lambda: dve(nc.vector.tensor_mul(p2, p4[:, :, 0::2], p4[:, :, 1::2]))
        yield lambda: dve(nc.vector.tensor_mul(w, p2[:, :, 0::2], p2[:, :, 1::2]))

    # Chains are processed in interleaved pairs: op N of wave b executes
    # between op N and N+1 of wave a, hiding the ~150ns sem-update
    # propagation of each producer->consumer hop behind the sibling
    # wave's op.  The last three waves run as solo groups in landing
    # order: pairing a landed wave with a still-streaming one stalls
    # the landed wave's ops on the partner's DMA sem (in-order DVE).
    groups = []
    k = 0
    while k < nw - 3:
        groups.append([k, k + 1])
        k += 2
    groups += [[k] for k in range(nw - 3, nw)]

    first_chain = True
    prev_cnt = {}
    for grp in groups:
        chains = {}
        for k in grp:
            sem, tgt = wave_done[k]
            nc.vector.wait_ge(sem, tgt)
            if first_chain:
                nc.vector.wait_ge(cq_sem, cq_tgt)
                first_chain = False
            chains[k] = chain_ops(k)
            prev_cnt[k] = None
        for step in range(7):
            for k in grp:
                if prev_cnt[k] is not None:
                    nc.vector.wait_ge(semDVE, prev_cnt[k])
                prev_cnt[k] = next(chains[k])()
                if step == 6:
                    w_ready[k] = (semDVE, prev_cnt[k])

    # --- PE: ordered PSUM accumulation, one wait per wave ---
    g = 0
    last_g = (IPP // R) - 1
    for k, nr in enumerate(WROWS):
        nc.tensor.wait_ge(*w_ready[k])
        for j in range(nr // R):
            mm = nc.tensor.matmul(
                acc.ap(),
                wts[k].ap()[:, j * R : (j + 1) * R, 0],
                vts[k].ap()[:, j * R : (j + 1) * R, 0:VD],
                start=(g == 0),
                stop=(g == last_g),
            )
            g += 1
    mm.then_inc(semPE, 1)

    # --- drain: PSUM -> HBM directly (or via SBUF with BMA_OUT=sbuf) ---
    if out_psum:
        nc.sync.wait_ge(semPE, 1)
        out_sem, out_tgt = dma(nc.sync, out.ap(), acc.ap(), nw)
    else:
        nc.vector.wait_ge(semPE, 1)
        res_done = dve(nc.vector.tensor_copy(res.ap(), acc.ap()))
        nc.sync.wait_ge(semDVE, res_done)
        out_sem, out_tgt = dma(nc.sync, out.ap(), res.ap(), nw)

    # The NEFF exit protocol zeroes every engine's entire semaphore file
    # between iterations (it is the dominant post-kernel cost), so the
    # explicit restore below is redundant; it only delays Sync's entry
    # into that mandatory epilogue.  BMA_CLEAR=1 restores it.
    if os.environ.get("BMA_CLEAR"):
        nc.sync.wait_ge(out_sem, out_tgt)
        for i, s in enumerate(dsems):
            if duses[i]:
                nc.sync.wait_ge(s, 16 * duses[i])
        nc.sync.wait_ge(semDVE, dcnt)
        nc.sync.wait_ge(semPE, 1)
        all_sems = dsems + [semDVE, semPE]
        lo = min(s.num for s in all_sems)
        hi = max(s.num for s in all_sems)
        nc.sync.drain(semaphore_range=range(lo, hi + 1))
        nc.sync.sem_clear(range(lo, hi + 1))

    nc.compile()
    return nc


def _get_nc(mode):
    key = (mode, os.environ.get("BMA_OUT", "sbuf"),
           os.environ.get("BMA_MAXSEM"))
    if key not in _CACHE:
        _CACHE[key] = _build_raw(mode)
    return _CACHE[key]


def run(x, query_addr, trace=False, mode=None):
    """Returns (output [B, 96] float32, BassKernelResults)."""
    from concourse.bass_utils import run_bass_kernel_spmd

    mode = mode or MM_MODE
    x = np.asarray(x)
    qa = int(np.asarray(query_addr))
    assert x.shape == (NCORES, S, D), x.shape

    qb = np.array([(qa >> k) & 1 for k in range(NBITS)], dtype=np.float32)
    cqv = np.ascontiguousarray(np.broadcast_to(qb, (P, NBITS)))

    nc = _get_nc(mode)
    in_maps = [
        {"x": np.ascontiguousarray(x[b], dtype=np.float32), "cq": cqv}
        for b in range(NCORES)
    ]
    if not trace:
        # A stray BASS_TRACE in the env would route run_bass_kernel_spmd
        # into the NTFF-hook path, which needs antenv.axon_hooks (absent
        # in this image unless test.py installs a shim).
        os.environ["BASS_NEVER_TRACE"] = "1"
    else:
        os.environ.pop("BASS_NEVER_TRACE", None)
    kres = run_bass_kernel_spmd(nc, in_maps, list(range(NCORES)), trace=trace)

    outs = []
    for r in kres.results:
        o = np.asarray(r["out"]).reshape(C, C, VD)
        outs.append(o[np.arange(C), np.arange(C)].sum(axis=0))
    return np.stack(outs).astype(np.float32), kres


def kernel(x, query_addr):
    return run(x, query_addr)[0]
